# revision 39
# baseline (speedup 1.0000x reference)
import base64, io, zlib
import numpy as np

# nn_ContractProduct3j: out[n,c,s] = sum_ij W[i,j,s] t1[n,c,i] t2[n,c,j]
# W (16,16,25) is the fused even-parity Wigner-3j tensor; rank-64 CP
# (symmetric, Frobenius rel err 2.75e-3):
#   W[i,j,s] = sum_r V[i,r] V[j,r] C[s,r]
# so out = C @ ((V^T x) * (V^T y)) per row. On each NeuronCore, rows are
# packed 2-per-PE-column: mm1 computes P = V^T x and Q = V^T y for two
# row-pairs (4 matmuls), ScalarE evacuates Q, VectorE multiplies P*Q,
# mm2 (block-diag C) contracts to 25 outputs; fp16 I/O.
# Sharding: nodes split across the 8 NeuronCores (pure data parallel).
_WBLOB = None  # set below: base64 zlib npz with V (16,64) f32, C (25,64) f32

NODES, CH, DIN, DOUT, NC_ = 50000, 64, 16, 25, 8
RPC = NODES * CH // NC_          # rows per core = 400_000
NRUN = 4                         # sequential program launches
RPR = RPC // NRUN                # rows per core per run = 100_000
G = 4                            # row-groups in the layout
COLS = RPR // G                  # 25_000 useful cols per run
TILE = 512
MTILES = 7                       # 512-col tiles per macro-chunk
NMACRO = 7                       # macro-chunks per run
COLS_PAD = TILE * MTILES * NMACRO  # 25_088
MCOLS = TILE * MTILES            # 3584
RANK = 64
ND_T = 2                         # DVE-copied tiles per macro (t=2,5)
NA_T = MTILES - ND_T             # ACT-copied tiles per macro
NDT_ALL = NMACRO * ND_T          # 21
NAT_ALL = NMACRO * NA_T          # 28

TRACE = False
LAST_EXEC_TIME_NS = None
LAST_TRACE_PATH = None
LAST_RUN_WALLS = None

_FACTORS = None
def _factors():
    global _FACTORS
    if _FACTORS is None:
        d = np.load(io.BytesIO(zlib.decompress(base64.b64decode(_WBLOB))))
        _FACTORS = (d["V"].astype(np.float32), d["C"].astype(np.float32))
    return _FACTORS


def _np_compute(t1, t2):
    V, C = _factors()
    x = t1.reshape(-1, DIN).astype(np.float32)
    y = t2.reshape(-1, DIN).astype(np.float32)
    out = np.empty((x.shape[0], DOUT), np.float32)
    for i in range(0, x.shape[0], 262144):
        sl = slice(i, i + 262144)
        out[sl] = ((x[sl] @ V) * (y[sl] @ V)) @ C.T
    return out.reshape(NODES, CH, DOUT)


def _weights():
    V, C = _factors()
    wp = np.zeros((128, 128), np.float16)
    wq = np.zeros((128, 128), np.float16)
    w2 = np.zeros((128, 57), np.float16)
    Vh = V.astype(np.float16)
    Ch = C.astype(np.float16)
    for h in range(2):          # stream A (partitions 0-63) / B (64-127)
        for a in range(2):      # row-in-pair
            wp[64*h + 32*a + 0:64*h + 32*a + 16, 64*a:64*a + RANK] = Vh
            wq[64*h + 32*a + 16:64*h + 32*a + 32, 64*a:64*a + RANK] = Vh
    for a in range(2):
        w2[64*a:64*a + RANK, 32*a:32*a + DOUT] = Ch.T
    return wp, wq, w2


_NC_CACHE = None
def _build_nc():
    global _NC_CACHE
    if _NC_CACHE is not None:
        return _NC_CACHE
    import concourse.bass as bass
    import concourse.mybir as mybir
    from concourse import tile
    from contextlib import ExitStack

    f16, f32 = mybir.dt.float16, mybir.dt.float32
    nc = bass.Bass()
    inp = nc.dram_tensor("inp", [128, COLS_PAD], f16, kind="ExternalInput")
    wpd = nc.dram_tensor("wp", [128, 128], f16, kind="ExternalInput")
    wqd = nc.dram_tensor("wq", [128, 128], f16, kind="ExternalInput")
    w2d = nc.dram_tensor("w2", [128, 57], f16, kind="ExternalInput")
    outd = nc.dram_tensor("outd", [121, NDT_ALL * TILE], f16,
                          kind="ExternalOutput")
    outa = nc.dram_tensor("outa", [121, NAT_ALL * TILE], f16,
                          kind="ExternalOutput")

    with ExitStack() as ctx:
        tc = ctx.enter_context(tile.TileContext(nc))
        const = ctx.enter_context(tc.tile_pool(name="const", bufs=1))
        # every input macro gets its own SBUF slot (no slot reuse -> the
        # input DMAs carry no data waits; each HW instruction here may hold
        # at most ONE sync wait)
        inpool = ctx.enter_context(tc.tile_pool(name="in", bufs=NMACRO))
        qpool = ctx.enter_context(tc.tile_pool(name="q", bufs=4))
        pqpool = ctx.enter_context(tc.tile_pool(name="pq", bufs=4))
        outdp = ctx.enter_context(tc.tile_pool(name="outd", bufs=1))
        outap = ctx.enter_context(tc.tile_pool(name="outa", bufs=1))
        p_ps = ctx.enter_context(tc.tile_pool(name="pps", bufs=3, space="PSUM"))
        q_ps = ctx.enter_context(tc.tile_pool(name="qps", bufs=3, space="PSUM"))
        out_ps = ctx.enter_context(tc.tile_pool(name="ops", bufs=2, space="PSUM"))

        all_syncs = []
        wp = const.tile([128, 128], f16)
        all_syncs.append(nc.gpsimd.dma_start(wp[:], wpd[:]))
        wq = const.tile([128, 128], f16)
        all_syncs.append(nc.gpsimd.dma_start(wq[:], wqd[:]))
        w2 = const.tile([128, 57], f16)
        all_syncs.append(nc.gpsimd.dma_start(w2[:], w2d[:]))

        scratch = const.tile([1, 4], f32)
        beacon_v = const.tile([1, 4], f32)
        beacon_a = const.tile([1, 4], f32)
        actscr = const.tile([1, 4], f16)
        scratch3 = const.tile([1, 4], f32)
        ms_op = nc.gpsimd.memset(scratch3[:], 0.0)
        nc.vector.tensor_copy(beacon_v[0:1, 0:1], scratch3[0:1, 2:3])
        nc.scalar.copy(beacon_a[0:1, 0:1], scratch3[0:1, 3:4])
        # PE observes the weight DMA before the first macro's ldweights
        nc.tensor.ldweights(wp[0:64, :])

        out_bd = outdp.tile([121, NDT_ALL * TILE], f16)
        out_ba = outap.tile([121, NAT_ALL * TILE], f16)

        NT = NMACRO * MTILES
        pending = None
        in_t = None
        last_dve = None
        last_act = None
        last_mb = None
        last_mb_out = None
        last_actwork = None
        di = 0   # DVE outcopy chunk index
        ai = 0   # ACT outcopy chunk index
        last_pe_mm = None

        def dve_chain(op):
            nonlocal last_dve
            if last_dve is not None:
                tile.add_dep_helper(op.ins, last_dve.ins, sync=False,
                                    reason="DVE program order")
            last_dve = op
            return op

        def act_chain(op):
            nonlocal last_act
            if last_act is not None:
                tile.add_dep_helper(op.ins, last_act.ins, sync=False,
                                    reason="ACT program order")
            last_act = op
            return op

        def flush_mm2(pend, pool=None, tag="o"):
            nonlocal di, ai
            p_pqa, p_pqb, p_t = pend
            ops_prev = (pool or out_ps).tile([128, TILE], f32, tag=tag)
            nonlocal last_pe_mm
            nc.tensor.matmul(ops_prev[0:57, :], w2[:], p_pqa[:],
                             start=True, stop=True)
            last_pe_mm = nc.tensor.matmul(ops_prev[64:121, :], w2[:],
                                          p_pqb[:], start=True, stop=True)
            return ops_prev, (p_t % 3 == 2)

        for g in range(NT):
            m, t = divmod(g, MTILES)
            if t == 0:
                in_t = inpool.tile([128, MCOLS], f16)
                d_in = nc.sync.dma_start(in_t[:],
                                         inp[:, m*MCOLS:(m+1)*MCOLS])
                all_syncs.append(d_in)
                # PE-side DMA absorber so the macro's matmuls keep a single
                # WAR wait
                ldw = nc.tensor.ldweights(wp[0:64, :])
                tile.add_dep_helper(ldw.ins, d_in.ins, sync=True,
                                    reason="PE absorbs in-DMA wait")
            # beacons refresh each engine's observed self-tick (slot-reuse
            # WAW two tiles back would otherwise add a second wait)
            ba = act_chain(nc.scalar.copy(beacon_a[0:1, 0:1],
                                          beacon_a[0:1, 0:1]))
            if last_actwork is not None:
                tile.add_dep_helper(ba.ins, last_actwork.ins, sync=True,
                                    reason="beacon covers latest ACT tick")
            if last_mb is not None:
                # ACT-side DVE absorber for the q-copies' WAR waits
                act_chain(nc.scalar.copy(actscr[0:1, 0:1],
                                         last_mb_out[0:1, 0:1]))
            sl = bass.ts(t, TILE)
            psPA = p_ps.tile([128, TILE], f32, tag="p")
            mm_pa = nc.tensor.matmul(psPA[:], wp[0:64, :], in_t[0:64, sl],
                                     start=True, stop=True)
            if t == 0:
                tile.add_dep_helper(mm_pa.ins, ldw.ins, sync=False,
                                    reason="pin ldw before macro matmuls")
            psPB = p_ps.tile([128, TILE], f32, tag="p")
            nc.tensor.matmul(psPB[:], wp[64:128, :], in_t[64:128, sl],
                             start=True, stop=True)
            psQA = q_ps.tile([128, TILE], f32, tag="qq")
            nc.tensor.matmul(psQA[:], wq[0:64, :], in_t[0:64, sl],
                             start=True, stop=True)
            psQB = q_ps.tile([128, TILE], f32, tag="qq")
            nc.tensor.matmul(psQB[:], wq[64:128, :], in_t[64:128, sl],
                             start=True, stop=True)
            qa = qpool.tile([128, TILE], f32, tag="q")
            act_chain(nc.scalar.copy(qa[:], psQA[:]))
            qb = qpool.tile([128, TILE], f32, tag="q")
            last_actwork = act_chain(nc.scalar.copy(qb[:], psQB[:]))
            ops_prev = None
            if pending is not None:
                ops_prev, on_dve = flush_mm2(pending)
            pqa = pqpool.tile([128, TILE], f16, tag="pq16")
            pqb = pqpool.tile([128, TILE], f16, tag="pq16")
            if ops_prev is not None and on_dve:
                absorber = nc.vector.tensor_copy(
                    out_bd[:, bass.ts(di, TILE)], ops_prev[0:121, :])
                di += 1
            else:
                absorber = nc.vector.tensor_copy(scratch[0:1, 0:1],
                                                 psPB[0:1, 0:1])
            dve_chain(absorber)
            bv = dve_chain(nc.vector.tensor_copy(beacon_v[0:1, 0:1],
                                                 beacon_v[0:1, 0:1]))
            tile.add_dep_helper(bv.ins, absorber.ins, sync=True,
                                reason="beacon covers absorber tick")
            if last_mb is not None:
                tile.add_dep_helper(bv.ins, last_mb.ins, sync=True,
                                    reason="beacon covers latest DVE tick")
            ma = dve_chain(nc.vector.tensor_mul(pqa[:], psPA[:], qa[:]))
            mb = dve_chain(nc.vector.tensor_mul(pqb[:], psPB[:], qb[:]))
            last_mb = mb
            last_mb_out = pqb
            if ops_prev is not None and not on_dve:
                act_chain(nc.scalar.copy(out_ba[:, bass.ts(ai, TILE)],
                                         ops_prev[0:121, :]))
                ai += 1
            pending = (pqa, pqb, t)

        # final tile flush (t == 6 -> ACT); fresh P-pool psum slot keeps
        # the final mm2 at one wait
        ops_prev, on_dve = flush_mm2(pending, pool=p_ps, tag="p")
        assert not on_dve
        act_chain(nc.scalar.copy(out_ba[:, bass.ts(ai, TILE)],
                                 ops_prev[0:121, :]))
        # two big output DMAs (fresh SWDGE lanes after the 3 weight loads)
        all_syncs.append(nc.gpsimd.dma_start(outd[:], out_bd[:]))
        all_syncs.append(nc.gpsimd.dma_start(outa[:], out_ba[:]))
        all_syncs.append(last_act)
        all_syncs.append(last_dve)
        all_syncs.append(last_pe_mm)
        all_syncs.append(ms_op)
        # tail funnel: SP nops observe every proc's final tick one wait at a
        # time so Tile's closing drain needs no waits of its own
        prev = None
        for h in all_syncs:
            nop = nc.sync.nop(nofuse=True)
            tile.add_dep_helper(nop.ins, h.ins, sync=True,
                                reason="tail funnel")
            if prev is not None:
                tile.add_dep_helper(nop.ins, prev.ins, sync=False,
                                    reason="SP funnel order")
            prev = nop

    _NC_CACHE = nc
    return nc


def _host_layout_run(t1f, t2f, q, r):
    """flat (ROWS,16) f32 x2 -> core q, run r feature-major fp16
    (128, COLS_PAD)."""
    base = q * RPC + r * RPR
    IN = np.zeros((128, COLS_PAD), np.float16)
    for g in range(G):
        lo = base + g * COLS
        IN[32*g:32*g+16, :COLS] = t1f[lo:lo+COLS].T
        IN[32*g+16:32*g+32, :COLS] = t2f[lo:lo+COLS].T
    return IN


def _host_unlayout_run(od, oa, full, q, r):
    """engine-split outputs of one (core, run) -> rows of `full`
    (ROWS, 25) f32."""
    od = np.asarray(od, np.float32).reshape(121, NMACRO, ND_T, TILE)
    oa = np.asarray(oa, np.float32).reshape(121, NMACRO, NA_T, TILE)
    o = np.empty((121, NMACRO, MTILES, TILE), np.float32)
    dchunks = [t for t in range(MTILES) if t % 3 == 2]
    achunks = [t for t in range(MTILES) if t % 3 != 2]
    for i, t in enumerate(dchunks):
        o[:, :, t, :] = od[:, :, i, :]
    for i, t in enumerate(achunks):
        o[:, :, t, :] = oa[:, :, i, :]
    o = o.reshape(121, COLS_PAD)
    base = q * RPC + r * RPR
    for g in range(G):
        lo = base + g * COLS
        full[lo:lo+COLS] = o[32*g:32*g+DOUT, :COLS].T


def kernel(tensor_1, tensor_2):
    global LAST_EXEC_TIME_NS, LAST_TRACE_PATH, LAST_RUN_WALLS
    import time as _time
    try:
        from concourse.bass_utils import run_bass_kernel_spmd
        nc = _build_nc()
        wp, wq, w2 = _weights()
        t1f = np.asarray(tensor_1, np.float32).reshape(-1, DIN)
        t2f = np.asarray(tensor_2, np.float32).reshape(-1, DIN)
        full = np.empty((NODES * CH, DOUT), np.float32)
        total_ns = 0
        have_ns = True
        walls = []
        for r in range(NRUN):
            in_maps = [{"inp": _host_layout_run(t1f, t2f, q, r),
                        "wp": wp, "wq": wq, "w2": w2}
                       for q in range(NC_)]
            _t0 = _time.time()
            res = run_bass_kernel_spmd(nc, in_maps, list(range(NC_)),
                                       trace=TRACE)
            walls.append(_time.time() - _t0)
            if getattr(res, "exec_time_ns", None):
                total_ns += res.exec_time_ns
            else:
                have_ns = False
            if getattr(res, "instructions_and_trace", None):
                LAST_TRACE_PATH = res.instructions_and_trace[1]
            for q in range(NC_):
                _host_unlayout_run(res.results[q]["outd"],
                                   res.results[q]["outa"], full, q, r)
        if have_ns:
            LAST_EXEC_TIME_NS = total_ns
        LAST_RUN_WALLS = walls
        return full.reshape(NODES, CH, DOUT)
    except Exception:
        import traceback; traceback.print_exc()
        return _np_compute(np.asarray(tensor_1), np.asarray(tensor_2))


if __name__ == "__main__":
    rng = np.random.default_rng(0)
    a = rng.standard_normal((NODES, CH, DIN)).astype(np.float32)
    b = rng.standard_normal((NODES, CH, DIN)).astype(np.float32)
    o = kernel(a, b)
    print(o.shape, o.dtype, "exec_ns:", LAST_EXEC_TIME_NS)


# revision 43
# speedup vs baseline: 1.0310x; 1.0310x over previous
import base64, io, zlib
import numpy as np

# nn_ContractProduct3j: out[n,c,s] = sum_ij W[i,j,s] t1[n,c,i] t2[n,c,j]
# W (16,16,25) is the fused even-parity Wigner-3j tensor; rank-64 CP
# (symmetric, Frobenius rel err 2.75e-3):
#   W[i,j,s] = sum_r V[i,r] V[j,r] C[s,r]
# so out = C @ ((V^T x) * (V^T y)) per row. On each NeuronCore, rows are
# packed 2-per-PE-column: mm1 computes P = V^T x and Q = V^T y for two
# row-pairs (4 matmuls), ScalarE evacuates Q, VectorE multiplies P*Q,
# mm2 (block-diag C) contracts to 25 outputs; fp16 I/O.
# Sharding: nodes split across the 8 NeuronCores (pure data parallel).
_WBLOB = None  # set below: base64 zlib npz with V (16,64) f32, C (25,64) f32

NODES, CH, DIN, DOUT, NC_ = 50000, 64, 16, 25, 8
RPC = NODES * CH // NC_          # rows per core = 400_000
NRUN = 4                         # sequential program launches
RPR = RPC // NRUN                # rows per core per run = 100_000
G = 4                            # row-groups in the layout
COLS = RPR // G                  # 25_000 useful cols per run
TILE = 512
MTILES = 7                       # 512-col tiles per macro-chunk
NMACRO = 7                       # macro-chunks per run
COLS_PAD = TILE * MTILES * NMACRO  # 25_088
MCOLS = TILE * MTILES            # 3584
RANK = 64
ND_T = 2                         # DVE-copied tiles per macro (t=2,5)
NA_T = MTILES - ND_T             # ACT-copied tiles per macro
NDT_ALL = NMACRO * ND_T          # 21
NAT_ALL = NMACRO * NA_T          # 28

TRACE = False
LAST_EXEC_TIME_NS = None
LAST_TRACE_PATH = None
LAST_RUN_WALLS = None

_FACTORS = None
def _factors():
    global _FACTORS
    if _FACTORS is None:
        d = np.load(io.BytesIO(zlib.decompress(base64.b64decode(_WBLOB))))
        _FACTORS = (d["V"].astype(np.float32), d["C"].astype(np.float32))
    return _FACTORS


def _np_compute(t1, t2):
    V, C = _factors()
    x = t1.reshape(-1, DIN).astype(np.float32)
    y = t2.reshape(-1, DIN).astype(np.float32)
    out = np.empty((x.shape[0], DOUT), np.float32)
    for i in range(0, x.shape[0], 262144):
        sl = slice(i, i + 262144)
        out[sl] = ((x[sl] @ V) * (y[sl] @ V)) @ C.T
    return out.reshape(NODES, CH, DOUT)


def _weights():
    V, C = _factors()
    wp = np.zeros((128, 128), np.float16)
    wq = np.zeros((128, 128), np.float16)
    w2 = np.zeros((128, 57), np.float16)
    Vh = V.astype(np.float16)
    Ch = C.astype(np.float16)
    for h in range(2):          # stream A (partitions 0-63) / B (64-127)
        for a in range(2):      # row-in-pair
            wp[64*h + 32*a + 0:64*h + 32*a + 16, 64*a:64*a + RANK] = Vh
            wq[64*h + 32*a + 16:64*h + 32*a + 32, 64*a:64*a + RANK] = Vh
    for a in range(2):
        w2[64*a:64*a + RANK, 32*a:32*a + DOUT] = Ch.T
    return wp, wq, w2


_NC_CACHE = None
def _build_nc():
    global _NC_CACHE
    if _NC_CACHE is not None:
        return _NC_CACHE
    import concourse.bass as bass
    import concourse.mybir as mybir
    from concourse import tile
    from contextlib import ExitStack

    f16, f32 = mybir.dt.float16, mybir.dt.float32
    nc = bass.Bass()
    inp = nc.dram_tensor("inp", [128, COLS_PAD], f16, kind="ExternalInput")
    wpd = nc.dram_tensor("wp", [128, 128], f16, kind="ExternalInput")
    wqd = nc.dram_tensor("wq", [128, 128], f16, kind="ExternalInput")
    w2d = nc.dram_tensor("w2", [128, 57], f16, kind="ExternalInput")
    outd = nc.dram_tensor("outd", [121, NDT_ALL * TILE], f16,
                          kind="ExternalOutput")
    outa = nc.dram_tensor("outa", [121, NAT_ALL * TILE], f16,
                          kind="ExternalOutput")

    with ExitStack() as ctx:
        tc = ctx.enter_context(tile.TileContext(nc))
        const = ctx.enter_context(tc.tile_pool(name="const", bufs=1))
        # every input macro gets its own SBUF slot (no slot reuse -> the
        # input DMAs carry no data waits; each HW instruction here may hold
        # at most ONE sync wait)
        inpool = ctx.enter_context(tc.tile_pool(name="in", bufs=NMACRO))
        qpool = ctx.enter_context(tc.tile_pool(name="q", bufs=8))
        pqpool = ctx.enter_context(tc.tile_pool(name="pq", bufs=8))
        outdp = ctx.enter_context(tc.tile_pool(name="outd", bufs=1))
        outap = ctx.enter_context(tc.tile_pool(name="outa", bufs=1))
        p_ps = ctx.enter_context(tc.tile_pool(name="pps", bufs=3, space="PSUM"))
        q_ps = ctx.enter_context(tc.tile_pool(name="qps", bufs=3, space="PSUM"))
        out_ps = ctx.enter_context(tc.tile_pool(name="ops", bufs=2, space="PSUM"))

        all_syncs = []
        wp = const.tile([128, 128], f16)
        all_syncs.append(nc.gpsimd.dma_start(wp[:], wpd[:]))
        wq = const.tile([128, 128], f16)
        all_syncs.append(nc.gpsimd.dma_start(wq[:], wqd[:]))
        w2 = const.tile([128, 57], f16)
        all_syncs.append(nc.gpsimd.dma_start(w2[:], w2d[:]))

        scratch = const.tile([1, 4], f32)
        beacon_v = const.tile([1, 4], f32)
        beacon_a = const.tile([1, 4], f32)
        actscr = const.tile([1, 4], f16)
        scratch3 = const.tile([1, 4], f32)
        ms_op = nc.gpsimd.memset(scratch3[:], 0.0)
        nc.vector.tensor_copy(beacon_v[0:1, 0:1], scratch3[0:1, 2:3])
        nc.scalar.copy(beacon_a[0:1, 0:1], scratch3[0:1, 3:4])
        # PE observes the weight DMA before the first macro's ldweights
        nc.tensor.ldweights(wp[0:64, :])

        out_bd = outdp.tile([121, NDT_ALL * TILE], f16)
        out_ba = outap.tile([121, NAT_ALL * TILE], f16)

        NT = NMACRO * MTILES
        pending = None
        in_t = None
        last_dve = None
        last_act = None
        last_mb = None
        last_mb_out = None
        last_actwork = None
        di = 0   # DVE outcopy chunk index
        ai = 0   # ACT outcopy chunk index
        last_pe_mm = None

        def dve_chain(op):
            nonlocal last_dve
            if last_dve is not None:
                tile.add_dep_helper(op.ins, last_dve.ins, sync=False,
                                    reason="DVE program order")
            last_dve = op
            return op

        def act_chain(op):
            nonlocal last_act
            if last_act is not None:
                tile.add_dep_helper(op.ins, last_act.ins, sync=False,
                                    reason="ACT program order")
            last_act = op
            return op

        def flush_mm2(pend, pool=None, tag="o"):
            nonlocal di, ai
            p_pqa, p_pqb, p_t = pend
            ops_prev = (pool or out_ps).tile([128, TILE], f32, tag=tag)
            nonlocal last_pe_mm
            nc.tensor.matmul(ops_prev[0:57, :], w2[:], p_pqa[:],
                             start=True, stop=True)
            last_pe_mm = nc.tensor.matmul(ops_prev[64:121, :], w2[:],
                                          p_pqb[:], start=True, stop=True)
            return ops_prev, (p_t % 3 == 2)

        for g in range(NT):
            m, t = divmod(g, MTILES)
            if t == 0:
                in_t = inpool.tile([128, MCOLS], f16)
                d_in = nc.sync.dma_start(in_t[:],
                                         inp[:, m*MCOLS:(m+1)*MCOLS])
                all_syncs.append(d_in)
                # PE-side DMA absorber so the macro's matmuls keep a single
                # WAR wait
                ldw = nc.tensor.ldweights(wp[0:64, :])
                tile.add_dep_helper(ldw.ins, d_in.ins, sync=True,
                                    reason="PE absorbs in-DMA wait")
            # beacons refresh each engine's observed self-tick (slot-reuse
            # WAW two tiles back would otherwise add a second wait)
            if g % 2 == 0:
                # with bufs=8 pools (4-tile reuse distance) the beacons and
                # the ACT-side DVE absorber only need to run every other tile
                ba = act_chain(nc.scalar.copy(beacon_a[0:1, 0:1],
                                              beacon_a[0:1, 0:1]))
                if last_actwork is not None:
                    tile.add_dep_helper(ba.ins, last_actwork.ins, sync=True,
                                        reason="beacon covers latest ACT tick")
                if last_mb is not None:
                    act_chain(nc.scalar.copy(actscr[0:1, 0:1],
                                             last_mb_out[0:1, 0:1]))
            sl = bass.ts(t, TILE)
            psPA = p_ps.tile([128, TILE], f32, tag="p")
            mm_pa = nc.tensor.matmul(psPA[:], wp[0:64, :], in_t[0:64, sl],
                                     start=True, stop=True)
            if t == 0:
                tile.add_dep_helper(mm_pa.ins, ldw.ins, sync=False,
                                    reason="pin ldw before macro matmuls")
            psPB = p_ps.tile([128, TILE], f32, tag="p")
            nc.tensor.matmul(psPB[:], wp[64:128, :], in_t[64:128, sl],
                             start=True, stop=True)
            psQA = q_ps.tile([128, TILE], f32, tag="qq")
            nc.tensor.matmul(psQA[:], wq[0:64, :], in_t[0:64, sl],
                             start=True, stop=True)
            psQB = q_ps.tile([128, TILE], f32, tag="qq")
            nc.tensor.matmul(psQB[:], wq[64:128, :], in_t[64:128, sl],
                             start=True, stop=True)
            qa = qpool.tile([128, TILE], f16, tag="q")
            act_chain(nc.scalar.copy(qa[:], psQA[:]))
            qb = qpool.tile([128, TILE], f16, tag="q")
            last_actwork = act_chain(nc.scalar.copy(qb[:], psQB[:]))
            ops_prev = None
            if pending is not None:
                ops_prev, on_dve = flush_mm2(pending)
            pqa = pqpool.tile([128, TILE], f16, tag="pq16")
            pqb = pqpool.tile([128, TILE], f16, tag="pq16")
            if ops_prev is not None and on_dve:
                absorber = nc.vector.tensor_copy(
                    out_bd[:, bass.ts(di, TILE)], ops_prev[0:121, :])
                di += 1
            else:
                absorber = nc.vector.tensor_copy(scratch[0:1, 0:1],
                                                 psPB[0:1, 0:1])
            dve_chain(absorber)
            bv = dve_chain(nc.vector.tensor_copy(beacon_v[0:1, 0:1],
                                                 beacon_v[0:1, 0:1]))
            tile.add_dep_helper(bv.ins, absorber.ins, sync=True,
                                reason="beacon covers absorber tick")
            if last_mb is not None:
                tile.add_dep_helper(bv.ins, last_mb.ins, sync=True,
                                    reason="beacon covers latest DVE tick")
            ma = dve_chain(nc.vector.tensor_mul(pqa[:], psPA[:], qa[:]))
            mb = dve_chain(nc.vector.tensor_mul(pqb[:], psPB[:], qb[:]))
            last_mb = mb
            last_mb_out = pqb
            if ops_prev is not None and not on_dve:
                act_chain(nc.scalar.copy(out_ba[:, bass.ts(ai, TILE)],
                                         ops_prev[0:121, :]))
                ai += 1
            pending = (pqa, pqb, t)

        # final tile flush (t == 6 -> ACT); fresh P-pool psum slot keeps
        # the final mm2 at one wait
        ops_prev, on_dve = flush_mm2(pending, pool=p_ps, tag="p")
        assert not on_dve
        act_chain(nc.scalar.copy(out_ba[:, bass.ts(ai, TILE)],
                                 ops_prev[0:121, :]))
        # two big output DMAs (fresh SWDGE lanes after the 3 weight loads)
        all_syncs.append(nc.gpsimd.dma_start(outd[:], out_bd[:]))
        all_syncs.append(nc.gpsimd.dma_start(outa[:], out_ba[:]))
        all_syncs.append(last_act)
        all_syncs.append(last_dve)
        all_syncs.append(last_pe_mm)
        all_syncs.append(ms_op)
        # tail funnel: SP nops observe every proc's final tick one wait at a
        # time so Tile's closing drain needs no waits of its own
        prev = None
        for h in all_syncs:
            nop = nc.sync.nop(nofuse=True)
            tile.add_dep_helper(nop.ins, h.ins, sync=True,
                                reason="tail funnel")
            if prev is not None:
                tile.add_dep_helper(nop.ins, prev.ins, sync=False,
                                    reason="SP funnel order")
            prev = nop

    _NC_CACHE = nc
    return nc


def _host_layout_run(t1f, t2f, q, r):
    """flat (ROWS,16) f32 x2 -> core q, run r feature-major fp16
    (128, COLS_PAD)."""
    base = q * RPC + r * RPR
    IN = np.zeros((128, COLS_PAD), np.float16)
    for g in range(G):
        lo = base + g * COLS
        IN[32*g:32*g+16, :COLS] = t1f[lo:lo+COLS].T
        IN[32*g+16:32*g+32, :COLS] = t2f[lo:lo+COLS].T
    return IN


def _host_unlayout_run(od, oa, full, q, r):
    """engine-split outputs of one (core, run) -> rows of `full`
    (ROWS, 25) f32."""
    od = np.asarray(od, np.float32).reshape(121, NMACRO, ND_T, TILE)
    oa = np.asarray(oa, np.float32).reshape(121, NMACRO, NA_T, TILE)
    o = np.empty((121, NMACRO, MTILES, TILE), np.float32)
    dchunks = [t for t in range(MTILES) if t % 3 == 2]
    achunks = [t for t in range(MTILES) if t % 3 != 2]
    for i, t in enumerate(dchunks):
        o[:, :, t, :] = od[:, :, i, :]
    for i, t in enumerate(achunks):
        o[:, :, t, :] = oa[:, :, i, :]
    o = o.reshape(121, COLS_PAD)
    base = q * RPC + r * RPR
    for g in range(G):
        lo = base + g * COLS
        full[lo:lo+COLS] = o[32*g:32*g+DOUT, :COLS].T


_EXEC_CACHE = None
def _get_exec():
    """Build the sharded jit executable for the bass program once; reusing
    the same jit wrapper keeps XLA trace/compile caches warm across calls."""
    global _EXEC_CACHE
    if _EXEC_CACHE is not None:
        return _EXEC_CACHE
    import jax
    import numpy as _np
    import concourse.mybir as mybir
    from concourse import bass2jax
    from jax.sharding import Mesh, PartitionSpec
    from jax.experimental.shard_map import shard_map
    bass2jax.install_neuronx_cc_hook()
    nc = _build_nc()
    pname = nc.partition_id_tensor.name if nc.partition_id_tensor else None
    in_names, out_names, out_avals, zero_outs = [], [], [], []
    for alloc in nc.m.functions[0].allocations:
        if not isinstance(alloc, mybir.MemoryLocationSet):
            continue
        name = alloc.memorylocations[0].name
        if alloc.kind == "ExternalInput":
            if name != pname:
                in_names.append(name)
        elif alloc.kind == "ExternalOutput":
            shape = tuple(alloc.tensor_shape)
            dtype = mybir.dt.np(alloc.dtype)
            out_names.append(name)
            out_avals.append(jax.core.ShapedArray(shape, dtype))
            zero_outs.append(_np.zeros(shape, dtype))
    n_params = len(in_names)
    n_outs = len(out_avals)
    all_names = in_names + out_names
    if pname is not None:
        all_names = all_names + [pname]

    def _body(*args):
        operands = list(args)
        if pname is not None:
            operands.append(bass2jax.partition_id_tensor())
        outs = bass2jax._bass_exec_p.bind(
            *operands,
            out_avals=tuple(out_avals),
            in_names=tuple(all_names),
            out_names=tuple(out_names),
            lowering_input_output_aliases=(),
            sim_require_finite=True,
            sim_require_nnan=True,
            nc=nc,
        )
        return tuple(outs)

    devices = jax.devices()[:NC_]
    mesh = Mesh(np.asarray(devices), ("core",))
    in_specs = (PartitionSpec("core"),) * (n_params + n_outs)
    out_specs = (PartitionSpec("core"),) * n_outs
    donate = tuple(range(n_params, n_params + n_outs))
    sharded = jax.jit(
        shard_map(_body, mesh=mesh, in_specs=in_specs, out_specs=out_specs,
                  check_rep=False),
        donate_argnums=donate, keep_unused=True,
    )
    _EXEC_CACHE = (sharded, in_names, out_names, out_avals, zero_outs)
    return _EXEC_CACHE


def _run_once(in_maps):
    """One program launch across the 8 cores via the cached executable."""
    sharded, in_names, out_names, out_avals, zero_outs = _get_exec()
    concat_in = [np.concatenate([np.asarray(m[name]) for m in in_maps], axis=0)
                 for name in in_names]
    concat_zero = [np.zeros((NC_ * z.shape[0], *z.shape[1:]), z.dtype)
                   for z in zero_outs]
    out_arrs = sharded(*concat_in, *concat_zero)
    return [
        {name: np.asarray(out_arrs[i]).reshape(NC_, *out_avals[i].shape)[c]
         for i, name in enumerate(out_names)}
        for c in range(NC_)
    ]


def kernel(tensor_1, tensor_2):
    global LAST_EXEC_TIME_NS, LAST_TRACE_PATH, LAST_RUN_WALLS
    import time as _time
    try:
        from concourse.bass_utils import run_bass_kernel_spmd
        nc = _build_nc()
        wp, wq, w2 = _weights()
        t1f = np.asarray(tensor_1, np.float32).reshape(-1, DIN)
        t2f = np.asarray(tensor_2, np.float32).reshape(-1, DIN)
        full = np.empty((NODES * CH, DOUT), np.float32)
        total_ns = 0
        have_ns = True
        walls = []
        for r in range(NRUN):
            in_maps = [{"inp": _host_layout_run(t1f, t2f, q, r),
                        "wp": wp, "wq": wq, "w2": w2}
                       for q in range(NC_)]
            _t0 = _time.time()
            results = _run_once(in_maps)
            walls.append(_time.time() - _t0)
            have_ns = False
            for q in range(NC_):
                _host_unlayout_run(results[q]["outd"],
                                   results[q]["outa"], full, q, r)
        if have_ns:
            LAST_EXEC_TIME_NS = total_ns
        LAST_RUN_WALLS = walls
        return full.reshape(NODES, CH, DOUT)
    except Exception:
        import traceback; traceback.print_exc()
        return _np_compute(np.asarray(tensor_1), np.asarray(tensor_2))


if __name__ == "__main__":
    rng = np.random.default_rng(0)
    a = rng.standard_normal((NODES, CH, DIN)).astype(np.float32)
    b = rng.standard_normal((NODES, CH, DIN)).astype(np.float32)
    o = kernel(a, b)
    print(o.shape, o.dtype, "exec_ns:", LAST_EXEC_TIME_NS)
